# revision 31
# baseline (speedup 1.0000x reference)
"""Trainium2 Bass kernel for the BDH-style sparse-attention model (v2).

Model (per reference): L=6 layers over T=1024 tokens, D=256, H=4 heads,
N=32768 neurons (NH=8192 per head), strict-causal unnormalized linear
attention with RoPE over the neuron dim, gated wide projection, encoder
contraction with residual layernorms, final vocab readout.

Sharding (8 NeuronCores): tensor-parallel over (head, neuron): core c owns
head h=c//2 and half of that head's neurons (4096), chosen as a contiguous
slice of the rope *pair* space so rotary stays core-local.

v2 structure (vs the v1 baseline), roughly 1.9x faster:
  * software-pipelined across layers: per-layer emission order is
    C(l,0) A(l+1,0) C(l,1) A(l+1,1) B(l+1,0) B(l+1,1), so every engine's
    in-order queue always holds next-stage work behind the trailing
    collectives instead of stalling on them.
  * pairwise a-reduce as AllGather + on-chip add (half the wire bytes of
    AllReduce, no CCE reduce pass); e-AllReduce in bf16.
  * collective staging DMAs ride the gpsimd (Pool) queue, ordered with
    the collectives themselves; readbacks issue from the consuming stage
    so they never head-block the next collective. Weight/trig streams
    stay on the SP queue with a prefetched head (emit_A_head) hoisted
    ahead of the gated B-stage streams.
  * x chunks are produced in rope-pair-interleaved order (PERM) so the
    rope starts immediately and the G matmuls stream behind it; G runs
    in PSUM groups of 5/3 ordered largest-first to ride the rope.
  * a-partials accumulate gt-major (one start..stop group at a time per
    PSUM bank - interleaved accumulation groups in one bank lose their
    has_written state to the next start).
  * e-matmul consumption deferred two chunks behind the y/gate producer
    so the PE never waits on the DVE gate chain.

All matmuls run in bf16 (fp32 PSUM accumulation); validated vs the fp32
reference at ~1.2e-2 max-rel (gate 2e-2).
"""

import numpy as np
import ml_dtypes

import concourse.bass as bass
import concourse.mybir as mybir
import concourse.tile as tile
from concourse import bacc
from concourse.bass_utils import run_bass_kernel_spmd

AF = mybir.ActivationFunctionType
ALU = mybir.AluOpType
F32 = mybir.dt.float32
BF16 = mybir.dt.bfloat16

NCORES = 8
D = 256
H = 4
L = 6
N = 32768
NH = N // H          # 8192
NLOC = NH // 2       # 4096 per-core neurons
HALF = NLOC // 2     # 2048 rope pairs per core
T = 1024
VOCAB = 256
ROPE_BASE = 10000.0
NCH = NLOC // 128    # 32 chunks of 128 neurons
NPAIR = NCH // 2     # 16 rope chunk pairs
TCN = 2              # t-chunks (pipeline stages)
TCW = T // TCN       # 512
TT = T // 128        # 8 global t-tiles
NG = NCH // 8        # 4 weight-stream groups of 8 chunks

REPLICA_PAIRS = [[0, 1], [2, 3], [4, 5], [6, 7]]
REPLICA_ALL = [list(range(NCORES))]
# x-production position -> chunk id: pair-interleaved so rope pair p
# (chunks p, NPAIR+p) can fire as soon as position 2p+1 is produced
PERM = [c for p in range(NPAIR) for c in (p, NPAIR + p)]


def build(nlayers: int = L, collectives: bool = True, dump=None):
    nc = bacc.Bacc(
        "TRN2", target_bir_lowering=False, debug=False,
        enable_asserts=False, num_devices=NCORES,
    )

    # ---- DRAM I/O ----
    wx_d = nc.dram_tensor("wx", [8, 128, 4, 2, 128], BF16, kind="ExternalInput")
    wy_d = nc.dram_tensor("wy", [8, 128, 4, 2, 128], BF16, kind="ExternalInput")
    enc_d = nc.dram_tensor("enc", [8, 128, 4, D], BF16, kind="ExternalInput")
    cs_d = nc.dram_tensor("cs", [TCN, 8, 128, 2, TCW], BF16, kind="ExternalInput")
    sn_d = nc.dram_tensor("sn", [TCN, 8, 128, 2, TCW], BF16, kind="ExternalInput")
    ro_d = nc.dram_tensor("ro", [128, 2, VOCAB], BF16, kind="ExternalInput")
    v0b_d = nc.dram_tensor("v0b", [128, TT, D], BF16, kind="ExternalInput")
    v0t_d = nc.dram_tensor("v0t", [128, 2, T], BF16, kind="ExternalInput")
    mask_d = nc.dram_tensor("maskd", [128, 128], BF16, kind="ExternalInput")
    ident_d = nc.dram_tensor("ident", [128, 128], BF16, kind="ExternalInput")
    out_d = nc.dram_tensor("out", [T, VOCAB], F32, kind="ExternalOutput")

    wx_ap, wy_ap = wx_d.ap(), wy_d.ap()
    cs_ap, sn_ap, enc_ap = cs_d.ap(), sn_d.ap(), enc_d.ap()

    with tile.TileContext(nc) as tc:
        with (
            tc.tile_pool(name="pers", bufs=1) as pers,
            tc.tile_pool(name="wxp", bufs=2) as wxp,
            tc.tile_pool(name="wyp", bufs=2) as wyp,
            tc.tile_pool(name="encp", bufs=2) as encp,
            tc.tile_pool(name="trig", bufs=2) as trig,
            tc.tile_pool(name="chbf", bufs=2) as chbf,
            tc.tile_pool(name="sbf", bufs=2) as sbf,
            tc.tile_pool(name="s32", bufs=2) as s32,
            tc.tile_pool(name="stat", bufs=2) as statp,
            tc.tile_pool(name="pbig", bufs=2, space="PSUM") as pbig,
            tc.tile_pool(name="pg", bufs=5, space="PSUM") as pgp,
            tc.tile_pool(name="pa", bufs=1, space="PSUM") as pap,
            tc.tile_pool(name="dram", bufs=2, space="DRAM") as dram,
        ):
            # ---- persistent SBUF ----
            ro = pers.tile([128, 2, VOCAB], BF16, tag="ro")
            maskd = pers.tile([128, 128], BF16, tag="maskd")
            ident = pers.tile([128, 128], BF16, tag="ident")
            xT = pers.tile([128, NCH, T], BF16, tag="xT")
            xrT = pers.tile([128, NCH, T], BF16, tag="xrT")
            S8 = pers.tile([128, TT, TCW], BF16, tag="S8")
            vb = pers.tile([128, TT, D], BF16, tag="vb")
            vt = pers.tile([128, 2, T], BF16, tag="vt")
            eps = pers.tile([128, 1], F32, tag="eps")
            nc.gpsimd.memset(eps[:], 1e-5)

            nc.sync.dma_start(ro[:], ro_d[:])
            nc.sync.dma_start(maskd[:], mask_d[:])
            nc.sync.dma_start(ident[:], ident_d[:])
            nc.sync.dma_start(vb[:], v0b_d[:])
            nc.sync.dma_start(vt[:], v0t_d[:])

            a_red = {}
            e_red = {}

            def ln_stats(src, nt, tag):
                """src [128, nt, D] -> (rstd, -mean*rstd) each [128, nt]."""
                bns = statp.tile([128, nt, 6], F32, tag=f"bns{tag}", bufs=2)
                agg = statp.tile([128, nt, 2], F32, tag=f"agg{tag}", bufs=2)
                for i in range(nt):
                    nc.vector.bn_stats(bns[:, i, :], src[:, i, :])
                    nc.vector.bn_aggr(agg[:, i, :], bns[:, i, :])
                std = statp.tile([128, nt], F32, tag=f"std{tag}", bufs=2)
                rstd = statp.tile([128, nt], F32, tag=f"rstd{tag}", bufs=2)
                nmr = statp.tile([128, nt], F32, tag=f"nmr{tag}", bufs=2)
                nc.scalar.activation(std[:], agg[:, :, 1], AF.Sqrt, bias=eps[:])
                nc.vector.reciprocal(rstd[:], std[:])
                nc.vector.scalar_tensor_tensor(
                    nmr[:], agg[:, :, 0], -1.0, rstd[:], ALU.mult, ALU.mult)
                return rstd, nmr

            def ln_apply(dst_ap_fn, src, nt, rstd, nmr):
                for i in range(nt):
                    nc.scalar.activation(
                        dst_ap_fn(i), src[:, i, :], AF.Identity,
                        bias=nmr[:, i:i + 1], scale=rstd[:, i:i + 1])

            def transpose_block(src_ap, dst_ap, name):
                ps = pbig.tile([128, 512], BF16, tag="pbig", name=name)
                nc.tensor.transpose(ps[:, 0:128], src_ap, ident[:])
                nc.scalar.activation(dst_ap, ps[:, 0:128], AF.Identity)

            # ================= stage emitters =================
            NPRE_W = 4    # wx groups prefetched (of 8)
            NPRE_T = 3    # trig pair-group tiles prefetched (of 8)

            def emit_A_head(l, tci):
                """Prefetch the head of stage A(l,tci)'s input stream.

                Issued during the preceding B stage so these DMAs sit ahead
                of the collective-gated wy/enc streams in the SP queue."""
                h = {"wxs": {}, "cs": {}, "sn": {}}
                for g in range(NPRE_W):
                    wxs = wxp.tile([128, 4, 2, 128], BF16, tag="wxs", bufs=5,
                                   name=f"wxs_{l}_{tci}_{g}")
                    nc.sync.dma_start(wxs[:], wx_ap[g])
                    h["wxs"][g] = wxs
                for pgi in range(NPRE_T):
                    co = trig.tile([128, 2, TCW], BF16, tag="cos", bufs=3,
                                   name=f"co_{l}_{tci}_{pgi}")
                    si = trig.tile([128, 2, TCW], BF16, tag="sin", bufs=3,
                                   name=f"si_{l}_{tci}_{pgi}")
                    nc.sync.dma_start(co[:], cs_ap[tci, pgi])
                    nc.sync.dma_start(si[:], sn_ap[tci, pgi])
                    h["cs"][pgi] = co
                    h["sn"][pgi] = si
                return h

            def emit_A(l, tci, head):
                """x = relu(v Wx); xr = rope(x); G blocks; fused a-partials;
                a AllGather (pairwise) issued."""
                t0c = tci * TCW
                sl = slice(t0c, t0c + TCW)
                # ---- x + relu ----
                for g in range(8):
                    if g in head["wxs"]:
                        wxs = head["wxs"][g]
                    else:
                        wxs = wxp.tile([128, 4, 2, 128], BF16, tag="wxs",
                                       bufs=5, name=f"wxs_{l}_{tci}_{g}")
                        nc.sync.dma_start(wxs[:], wx_ap[g])
                    for j in range(4):
                        cc = PERM[4 * g + j]
                        ps = pbig.tile([128, TCW], F32, tag="pbig",
                                       name=f"px_{l}_{tci}_{cc}")
                        for dc in range(2):
                            nc.tensor.matmul(
                                ps[:], wxs[:, j, dc, :], vt[:, dc, sl],
                                start=(dc == 0), stop=(dc == 1))
                        nc.scalar.activation(xT[:, cc, sl], ps[:], AF.Relu)
                # ---- rope ----
                for pgi in range(8):
                    if pgi in head["cs"]:
                        co, si = head["cs"][pgi], head["sn"][pgi]
                    else:
                        co = trig.tile([128, 2, TCW], BF16, tag="cos", bufs=3,
                                       name=f"co_{l}_{tci}_{pgi}")
                        si = trig.tile([128, 2, TCW], BF16, tag="sin", bufs=3,
                                       name=f"si_{l}_{tci}_{pgi}")
                        nc.sync.dma_start(co[:], cs_ap[tci, pgi])
                        nc.sync.dma_start(si[:], sn_ap[tci, pgi])
                    for k in range(2):
                        c = 2 * pgi + k
                        p2 = chbf.tile([128, TCW], BF16, tag="p2",
                                       name=f"p2_{l}_{tci}_{c}")
                        nc.vector.tensor_tensor(
                            xrT[:, c, sl], xT[:, c, sl], co[:, k, :], ALU.mult)
                        nc.vector.tensor_tensor(
                            p2[:], xT[:, NPAIR + c, sl], si[:, k, :], ALU.mult)
                        nc.vector.tensor_tensor(
                            xrT[:, c, sl], xrT[:, c, sl], p2[:], ALU.subtract)
                        p3 = chbf.tile([128, TCW], BF16, tag="p2",
                                       name=f"p3_{l}_{tci}_{c}")
                        nc.vector.tensor_tensor(
                            xrT[:, NPAIR + c, sl], xT[:, NPAIR + c, sl],
                            co[:, k, :], ALU.mult)
                        nc.vector.tensor_tensor(
                            p3[:], xT[:, c, sl], si[:, k, :], ALU.mult)
                        nc.vector.tensor_tensor(
                            xrT[:, NPAIR + c, sl], xrT[:, NPAIR + c, sl],
                            p3[:], ALU.add)

                # ---- G blocks fused with a-partials ----
                gts = list(range(4 * tci, 4 * tci + 4))
                sts = list(range(4 * (tci + 1)))
                groups = [sts[:5], sts[5:]] if len(sts) > 5 else [sts]
                cc_order = [c for p in range(NPAIR) for c in (p, NPAIR + p)]
                for grp in groups:
                    pgs, geom = {}, {}
                    for st in grp:
                        tg0 = max(st * 128, t0c)
                        nw = t0c + TCW - tg0
                        geom[st] = (tg0, nw)
                        pgs[st] = pgp.tile([128, TCW], F32, tag="pg",
                                           name=f"pg_{l}_{tci}_{st}")
                    for ci, cc in enumerate(cc_order):
                        for st in grp:
                            tg0, nw = geom[st]
                            nc.tensor.matmul(
                                pgs[st][:, :nw],
                                xrT[:, cc, st * 128:(st + 1) * 128],
                                xrT[:, cc, tg0:tg0 + nw],
                                start=(ci == 0), stop=(ci == NCH - 1))
                    for st in grp:
                        tg0, nw = geom[st]
                        pgt = pgs[st]
                        if tg0 == st * 128:
                            nc.vector.tensor_tensor(
                                S8[:, st, 0:128], pgt[:, 0:128], maskd[:],
                                ALU.mult)
                            if nw > 128:
                                nc.scalar.activation(
                                    S8[:, st, 128:nw], pgt[:, 128:nw],
                                    AF.Identity)
                        else:
                            nc.scalar.activation(
                                S8[:, st, :nw], pgt[:, :nw], AF.Identity)
                # gt-major so each accumulation group's start..stop never
                # interleaves with another group in the same PSUM bank
                a_loc = sbf.tile([128, 4, D], BF16, tag="a_loc",
                                 name=f"a_loc_{l}_{tci}")
                for gt in gts:
                    pa = pap.tile([128, D], F32, tag="pa",
                                  name=f"pa_{l}_{gt}")
                    for st in range(gt + 1):
                        tg0 = max(st * 128, t0c)
                        off = gt * 128 - tg0
                        nc.tensor.matmul(
                            pa[:], S8[:, st, off:off + 128],
                            vb[:, st, :],
                            start=(st == 0), stop=(st == gt))
                    nc.scalar.activation(a_loc[:, gt - 4 * tci, :], pa[:],
                                         AF.Identity)

                # ---- pairwise a reduce: AllGather + local add ----
                if collectives:
                    ain = dram.tile([128, 4, D], BF16, tag="ain",
                                    name=f"ain_{l}_{tci}")
                    aout = dram.tile([2, 128, 4, D], BF16, tag="aout",
                                     name=f"aout_{l}_{tci}")
                    nc.gpsimd.dma_start(ain[:], a_loc[:])
                    if collectives == "dma":
                        nc.gpsimd.dma_start(aout[0], ain[:])
                        nc.gpsimd.dma_start(aout[1], ain[:])
                    else:
                        nc.gpsimd.collective_compute(
                            "AllGather", ALU.bypass, replica_groups=REPLICA_PAIRS,
                            ins=[ain.opt()], outs=[aout.opt()])
                    a_red[(l, tci)] = (a_loc, aout)
                else:
                    a_red[(l, tci)] = a_loc

            def emit_B(l, tci):
                """lnA; y = relu(lnA Wy) * x; e-partial = enc^T y;
                e AllReduce (all cores, bf16) issued."""
                t0c = tci * TCW
                sl = slice(t0c, t0c + TCW)
                if collectives:
                    a_loc, aout = a_red[(l, tci)]
                    # overwrite a_loc with shard 0, land shard 1 beside it,
                    # sum in place: ar = shard0 + shard1
                    apr = sbf.tile([128, 4, D], BF16, tag="apeer",
                                   name=f"apeer_{l}_{tci}")
                    nc.gpsimd.dma_start(a_loc[:], aout[0])
                    nc.gpsimd.dma_start(apr[:], aout[1])
                    nc.vector.tensor_tensor(
                        apr[:], a_loc[:], apr[:], ALU.add)
                    ar = apr
                    if dump == "a" and l == 0:
                        for i in range(4):
                            gt = 4 * tci + i
                            nc.gpsimd.dma_start(
                                out_d[gt * 128:(gt + 1) * 128, :],
                                apr[:, i, :])
                else:
                    ar = a_red[(l, tci)]
                rstd, nmr = ln_stats(ar, 4, "a")
                lnA = sbf.tile([128, 4, D], BF16, tag="lnA", bufs=1,
                               name=f"lnA_{l}_{tci}")
                ln_apply(lambda i: lnA[:, i, :], ar, 4, rstd, nmr)
                lat = sbf.tile([128, 2, TCW], BF16, tag="lnAT", bufs=1,
                               name=f"lnAT_{l}_{tci}")
                for i in range(4):
                    for dc in range(2):
                        transpose_block(
                            lnA[:, i, dc * 128:(dc + 1) * 128],
                            lat[:, dc, i * 128:(i + 1) * 128],
                            f"ptA_{l}_{tci}_{i}_{dc}")

                pe0 = pgp.tile([128, TCW], F32, tag="pg", name=f"pe0_{l}_{tci}")
                pe1 = pgp.tile([128, TCW], F32, tag="pg", name=f"pe1_{l}_{tci}")
                pend = []

                def emit_pe(c, yc):
                    for dc, pe in ((0, pe0), (1, pe1)):
                        nc.tensor.matmul(
                            pe[:], ecs[c // 4][:, c % 4, dc * 128:(dc + 1) * 128],
                            yc[:], start=(c == 0), stop=(c == NCH - 1))

                ecs = {}
                for g in range(8):
                    wys = wyp.tile([128, 4, 2, 128], BF16, tag="wys",
                                   name=f"wys_{l}_{tci}_{g}")
                    nc.sync.dma_start(wys[:], wy_ap[g])
                    ec = encp.tile([128, 4, D], BF16, tag="enc", bufs=3,
                                   name=f"ec_{l}_{tci}_{g}")
                    nc.sync.dma_start(ec[:], enc_ap[g])
                    ecs[g] = ec
                    for j in range(4):
                        c = 4 * g + j
                        py = pbig.tile([128, TCW], F32, tag="pbig",
                                       name=f"py_{l}_{tci}_{c}")
                        for dc in range(2):
                            nc.tensor.matmul(
                                py[:], wys[:, j, dc, :], lat[:, dc, :],
                                start=(dc == 0), stop=(dc == 1))
                        yc = chbf.tile([128, TCW], BF16, tag="yc", bufs=4,
                                       name=f"yc_{l}_{tci}_{c}")
                        nc.vector.scalar_tensor_tensor(
                            yc[:], py[:], 0.0, xT[:, c, sl], ALU.max, ALU.mult)
                        pend.append((c, yc))
                        if len(pend) > 2:
                            emit_pe(*pend.pop(0))
                for c, yc in pend:
                    emit_pe(c, yc)

                # evac e^T (bf16), AllReduce in that layout
                eT = s32.tile([128, 2, TCW], BF16, tag="eT", bufs=1,
                              name=f"eT_{l}_{tci}")
                nc.scalar.activation(eT[:, 0, :], pe0[:], AF.Identity)
                nc.scalar.activation(eT[:, 1, :], pe1[:], AF.Identity)
                if collectives:
                    ein = dram.tile([128, 2, TCW], BF16, tag="ein",
                                    name=f"ein_{l}_{tci}")
                    eout = dram.tile([128, 2, TCW], BF16, tag="eout",
                                     name=f"eout_{l}_{tci}")
                    nc.gpsimd.dma_start(ein[:], eT[:])
                    if collectives == "dma":
                        nc.gpsimd.dma_start(eout[:], ein[:])
                    else:
                        nc.gpsimd.collective_compute(
                            "AllReduce", ALU.add, replica_groups=REPLICA_ALL,
                            ins=[ein.opt()], outs=[eout.opt()])
                    e_red[(l, tci)] = eout
                else:
                    e_red[(l, tci)] = eT

            def emit_C(l, tci):
                """v = ln(v + ln(e)); refresh vb rows and vt columns."""
                if collectives:
                    eout = e_red[(l, tci)]
                    ert = s32.tile([128, 2, TCW], BF16, tag="ert", bufs=1,
                                   name=f"ert_{l}_{tci}")
                    nc.gpsimd.dma_start(ert[:], eout[:])
                else:
                    ert = e_red[(l, tci)]
                er = sbf.tile([128, 4, D], BF16, tag="e_red",
                              name=f"e_red_{l}_{tci}")
                for i in range(4):
                    for dc in range(2):
                        transpose_block(
                            ert[:, dc, i * 128:(i + 1) * 128],
                            er[:, i, dc * 128:(dc + 1) * 128],
                            f"ptE_{l}_{tci}_{i}_{dc}")
                rstd, nmr = ln_stats(er, 4, "e")
                lnE = s32.tile([128, 4, D], BF16, tag="lnE", bufs=1,
                               name=f"lnE_{l}_{tci}")
                ln_apply(lambda i: lnE[:, i, :], er, 4, rstd, nmr)
                nc.vector.tensor_tensor(
                    lnE[:], vb[:, 4 * tci:4 * tci + 4, :], lnE[:], ALU.add)
                rstd2, nmr2 = ln_stats(lnE, 4, "v")
                for i in range(4):
                    gt = 4 * tci + i
                    nc.scalar.activation(
                        vb[:, gt, :], lnE[:, i, :], AF.Identity,
                        bias=nmr2[:, i:i + 1], scale=rstd2[:, i:i + 1])
                    for dc in range(2):
                        transpose_block(
                            vb[:, gt, dc * 128:(dc + 1) * 128],
                            vt[:, dc, gt * 128:(gt + 1) * 128],
                            f"ptV_{l}_{tci}_{gt}_{dc}")

            # ================= pipelined emission =================
            hA = {}
            hA[(0, 0)] = emit_A_head(0, 0)
            emit_A(0, 0, hA[(0, 0)])
            hA[(0, 1)] = emit_A_head(0, 1)
            emit_A(0, 1, hA[(0, 1)])
            emit_B(0, 0)
            emit_B(0, 1)
            for l in range(nlayers):
                if l + 1 < nlayers:
                    hA[(l + 1, 0)] = emit_A_head(l + 1, 0)
                emit_C(l, 0)
                if l + 1 < nlayers:
                    emit_A(l + 1, 0, hA[(l + 1, 0)])
                    hA[(l + 1, 1)] = emit_A_head(l + 1, 1)
                emit_C(l, 1)
                if l + 1 < nlayers:
                    emit_A(l + 1, 1, hA[(l + 1, 1)])
                    emit_B(l + 1, 0)
                    emit_B(l + 1, 1)

            # ---------------- readout ----------------
            for gt in range(TT if not dump else 0):
                ps = pbig.tile([128, VOCAB], F32, tag="pbig", name=f"pro_{gt}")
                for dc in range(2):
                    nc.tensor.matmul(
                        ps[:], vt[:, dc, gt * 128:(gt + 1) * 128],
                        ro[:, dc, :],
                        start=(dc == 0), stop=(dc == 1),
                    )
                ob = s32.tile([128, VOCAB], F32, tag="lnE", bufs=1, name=f"ob_{gt}")
                nc.vector.tensor_copy(ob[:], ps[:])
                nc.sync.dma_start(out_d[gt * 128:(gt + 1) * 128, :], ob[:])

    nc.compile()
    return nc


def prep_inputs(inputs):
    """Full inputs -> per-core in_maps (host-side shard + precompute)."""
    bf = ml_dtypes.bfloat16
    idx = np.asarray(inputs["idx"], dtype=np.int32)
    wte = np.asarray(inputs["wte"], dtype=np.float32)
    enc = np.asarray(inputs["encoder"], dtype=np.float32)
    dx = np.asarray(inputs["decoder_x"], dtype=np.float32)
    dy = np.asarray(inputs["decoder_y"], dtype=np.float32)
    ro = np.asarray(inputs["readout"], dtype=np.float32)

    # embedding + initial layernorm (host)
    v0 = wte[idx[0]]
    m = v0.mean(-1, keepdims=True)
    va = v0.var(-1, keepdims=True)
    v0 = ((v0 - m) / np.sqrt(va + 1e-5)).astype(np.float32)  # [T, D]
    v0b = np.ascontiguousarray(
        v0.reshape(TT, 128, D).transpose(1, 0, 2)).astype(bf)
    v0t = np.ascontiguousarray(
        v0.T.reshape(2, 128, T).transpose(1, 0, 2)).astype(bf)

    half_g = NH // 2
    inv = 1.0 / (ROPE_BASE ** (np.arange(half_g, dtype=np.float32) / half_g))
    tarr = np.arange(T, dtype=np.float32)

    mask = np.triu(np.ones((128, 128), np.float32), k=1).astype(bf)
    ident = np.eye(128, dtype=np.float32).astype(bf)
    ro_arr = np.ascontiguousarray(
        ro.reshape(2, 128, VOCAB).transpose(1, 0, 2)).astype(bf)

    in_maps = []
    for c in range(NCORES):
        h, p = c // 2, c % 2
        j0, j1 = p * HALF, (p + 1) * HALF
        cols = np.r_[j0:j1, half_g + j0:half_g + j1]
        wx_c = dx[h][:, cols]   # [256, 4096]
        wy_c = dy[h][:, cols]
        enc_c = enc[h * NH:(h + 1) * NH][cols]  # [4096, 256]

        # [256, 4096] -> [8, 128, 4, 2, 128]:
        # d = dc*128 + part; chunk at position 4g+j is PERM[4g+j]
        wx_arr = np.ascontiguousarray(
            wx_c.reshape(2, 128, 32, 128)[:, :, PERM, :]
            .reshape(2, 128, 8, 4, 128).transpose(2, 1, 3, 0, 4)
        ).astype(bf)
        wy_arr = np.ascontiguousarray(
            wy_c.reshape(2, 128, 8, 4, 128).transpose(2, 1, 3, 0, 4)
        ).astype(bf)
        # [4096, 256] -> [8, 128, 4, D]: n = 128*(4g + j) + part
        enc_arr = np.ascontiguousarray(
            enc_c.reshape(8, 4, 128, D).transpose(0, 2, 1, 3)).astype(bf)

        ang = tarr[:, None] * inv[None, j0:j1]      # [T, 2048]
        cos = np.cos(ang).T.astype(np.float32)      # [2048, T]
        sin = np.sin(ang).T.astype(np.float32)
        # [2048, T] -> [TCN, 8, 128, 2, TCW]: row = 128*(2*pg + k) + part
        cs_arr = np.ascontiguousarray(
            cos.reshape(8, 2, 128, TCN, TCW).transpose(3, 0, 2, 1, 4)
        ).astype(bf)
        sn_arr = np.ascontiguousarray(
            sin.reshape(8, 2, 128, TCN, TCW).transpose(3, 0, 2, 1, 4)
        ).astype(bf)

        in_maps.append({
            "wx": wx_arr, "wy": wy_arr, "enc": enc_arr,
            "cs": cs_arr, "sn": sn_arr, "ro": ro_arr,
            "v0b": v0b, "v0t": v0t, "maskd": mask, "ident": ident,
        })
    return in_maps


_NC_CACHE = {}


def get_nc(nlayers: int = L):
    if nlayers not in _NC_CACHE:
        _NC_CACHE[nlayers] = build(nlayers)
    return _NC_CACHE[nlayers]


def kernel(**inputs) -> np.ndarray:
    nc = get_nc()
    in_maps = prep_inputs(inputs)
    res = run_bass_kernel_spmd(nc, in_maps, core_ids=list(range(NCORES)))
    out = res.results[0]["out"].astype(np.float32)
    return out.reshape(1, T, VOCAB)
